# revision 35
# baseline (speedup 1.0000x reference)
"""Trainium2 Bass/Tile kernel: two chained VALID 3x3 convolutions.

    x  [N,3,256,256] --conv(w1)--> h [N,64,254,254] --conv(w2)--> out [N,128,252,252]

Data-parallel over 8 NeuronCores: batch N=16 -> 2 images per core, conv
weights replicated.

Perf structure (v5):

conv2 runs as 6 K=128 matmul passes per 2-row output chunk against a
doubled SBUF buffer H (partitions 0:64 = h, 64:128 = h shifted down one
row):

  wp[dj] @ H(t,   dj) -> taps (0,dj)+(1,dj)   (3 pair passes)
  ws[dj] @ H(t+1, dj) -> tap  (2,dj)          (3 passes, top-half weights
                                               zero so K stays 128)

Everything is tile-config (128,128), so the PE never pays the 64<->128
row-config switch the v2 kernel paid per block.

conv1 runs as 2 concurrent row-tiled K=27 matmuls (PE row groups (0,0)
and (32,0)).  The im2col buffer B1 is a flat per-partition byte stream
holding full 512 B x rows; the (di,dj) tap shift is a per-partition
byte offset, so each 3-partition load is ONE contiguous ~19.5 KB DMA
packet from HBM (38 rows of x at full width).

Copies (conv1 PSUM P1 holds h twice via column-duplicated conv1
weights, keeping everything partition-aligned):
  DVE/ACT (alternating per chunk): P1[0:64] -> H[0:64]   (the h rows)
  DMA (strip pieces):  H[0:64] rows+1 -> H[64:128]       (the row shift)
  DVE/ACT (alternating per chunk): conv2 PSUM -> OS bf16

conv2 output is staged in SBUF as bf16 and DMA'd to HBM in 6-row
pieces split into 4 per-channel-group chains (parallel DMA engines);
the host upcasts to f32.  Lookahead is one strip: phase s = conv2(s) +
conv1(s+1) bursts + im2col(s+2) loads.
"""

from contextlib import ExitStack

import ml_dtypes
import numpy as np

import concourse.bass as bass
import concourse.mybir as mybir
import concourse.tile as tile
import concourse.bass_utils as bass_utils
from concourse import bacc

N_CORES = 8
FULL_N = 16
C0, C1, C2 = 3, 64, 128

H0 = W0 = 256
H1 = W1 = 254
H2 = W2 = 252
TY = 36                                  # conv2 output rows per strip
NPC = FULL_N // N_CORES                  # images per core
SPI = H2 // TY                           # strips per image (7)
NSTRIPS = NPC * SPI                      # strips per core (14)
NC1 = (TY + 2) // 2                      # conv1 2-row chunks per strip (19)
NC2 = TY // 2                            # conv2 2-row chunks per strip (18)
BLK = 3                                  # conv2 chunks per block
NB = NC2 // BLK                          # conv2 blocks per strip (6)
NG = (NC1 + 1) // 2                      # conv1 pair-groups per strip (10)

B1_ROWS = 38                             # x rows loaded per im2col partition
B1_FLAT = 39 * W0                        # flat B1 elems/partition (spill pad)

MODE = "bf16"


def _mm_dt():
    return mybir.dt.bfloat16 if MODE == "bf16" else mybir.dt.float32r


def _emit(ctx: ExitStack, tc: tile.TileContext, out, x, w1q, wp, ws, mm_dt):
    nc = tc.nc
    f32 = mybir.dt.float32

    wpool = ctx.enter_context(tc.tile_pool(name="weights", bufs=1))
    b1pool = ctx.enter_context(tc.tile_pool(name="b1", bufs=4))
    hpool = ctx.enter_context(tc.tile_pool(name="h", bufs=3))
    opool = ctx.enter_context(tc.tile_pool(name="o2", bufs=4))
    ps1 = ctx.enter_context(tc.tile_pool(name="ps1", bufs=2, space="PSUM"))
    ps2 = ctx.enter_context(tc.tile_pool(name="ps2", bufs=4, space="PSUM"))

    # conv1 weights: [27, 128] (output cols duplicated), one copy per PE
    # 32-row group so 2 chunk matmuls run concurrently via row tiling.
    w1sb = wpool.tile([64, 128], mm_dt, tag="w1")
    for q in range(2):
        nc.sync.dma_start(w1sb[32 * q:32 * q + 27, :], w1q)
    wp_sb, ws_sb = [], []
    for dj in range(3):
        wpt = wpool.tile([128, C2], mm_dt, tag=f"wp{dj}")
        nc.sync.dma_start(wpt[:], wp[dj])
        wp_sb.append(wpt)
        wst = wpool.tile([128, C2], mm_dt, tag=f"ws{dj}")
        nc.sync.dma_start(wst[:], ws[dj])
        ws_sb.append(wst)

    def strip_of(s):
        n, k = divmod(s, SPI)
        return n, k * TY

    B1_tiles, H_tiles = {}, {}

    def emit_b1_alloc(s):
        B1_tiles[s] = b1pool.tile([64, B1_FLAT], mm_dt, tag="b1",
                                  name=f"B1_{s}")

    def emit_im2col(s, taps=range(9)):
        """Flat im2col: partition 32q + 3*(3di+dj) + c holds x rows
        y0+di..y0+di+38 as full 512B rows at element offset (2-dj); the
        conv1 moving AP then reads tap (di,dj) at uniform offset col 2.
        Each tap is ONE contiguous ~19.5KB HBM read per partition,
        written to both 32-partition quadrants by one dma_start; taps
        are issued a few per conv2 block, 2 phases ahead, because HBM
        reads are served by only ~3 DMA engines and dma_start issue
        costs ~700ns on the issuing engine."""
        n, y0 = strip_of(s)
        B1 = B1_tiles[s]
        for t9 in taps:
            di, dj = divmod(t9, 3)
            off = 2 - dj
            src = x[n, :, y0 + di:y0 + di + B1_ROWS, :]
            for q in range(2):
                p = 32 * q + 3 * t9
                nc.sync.dma_start(B1[p:p + 3, off:off + B1_ROWS * W0], src)

    def emit_conv1_pair(s, g):
        """conv1 chunks 2g..2g+1 as concurrent row-tiled K=27 matmuls
        into ONE 2-bank PSUM tile (chunk i at rows 2i, each matmul's
        512-element output staying inside one bank).  The moving AP
        takes the full 256-col window; PSUM cols 0:2 catch garbage from
        the flat layout's leading bytes and are never evicted.

        Both H halves evict from this tile at 4-row granularity:
        DVE does H[0:64] (h rows 4g..4g+4), ACT does H[64:128] at a
        -1-row offset (the row shift) -- no DMA on the critical path."""
        if g == 0:
            H_tiles[s] = hpool.tile([128, B1_ROWS, W1], mm_dt, tag="h",
                                    name=f"H{s}")
        H = H_tiles[s]
        B1v = B1_tiles[s].rearrange("p (r c) -> p r c", c=W0)
        nch = min(2, NC1 - 2 * g)
        P1 = ps1.tile([128, 4, W0], f32, tag="p1")
        for i in range(nch):
            j = 2 * g + i
            nc.tensor.matmul(
                P1[:, 2 * i:2 * i + 2, :], w1sb[32 * i:32 * i + 27, :],
                B1v[32 * i:32 * i + 27, 2 * j:2 * j + 2, :],
                start=True, stop=True, tile_position=(32 * i, 0))
        r = 4 * g
        nr = 2 * nch
        nc.vector.tensor_copy(H[0:C1, r:r + nr, :], P1[0:C1, 0:nr, 2:W0])
        if g == 0:
            nc.scalar.copy(H[C1:128, 0:nr - 1, :], P1[C1:128, 1:nr, 2:W0])
        else:
            nc.scalar.copy(H[C1:128, r - 1:r + nr - 1, :],
                           P1[C1:128, 0:nr, 2:W0])

    def emit_conv2_block(s, k):
        n, y0 = strip_of(s)
        H = H_tiles[s]
        OS = opool.tile([C2, 2 * BLK, W2], mm_dt, tag="os")
        for c in range(BLK):
            cc = BLK * k + c
            t = 2 * cc
            P2 = ps2.tile([C2, 2, W2], f32, tag="p2", name=f"P2_{c}")
            for dj in range(3):
                nc.tensor.matmul(P2[:], wp_sb[dj][:],
                                 H[:, t:t + 2, dj:dj + W2],
                                 start=(dj == 0), stop=False,
                                 skip_group_check=True)
            for dj in range(3):
                nc.tensor.matmul(P2[:], ws_sb[dj][:],
                                 H[:, t + 1:t + 3, dj:dj + W2],
                                 start=False, stop=(dj == 2),
                                 skip_group_check=True)
            if cc % 2 == 0:
                nc.vector.tensor_copy(OS[:, 2 * c:2 * c + 2, :], P2[:])
            else:
                nc.scalar.copy(OS[:, 2 * c:2 * c + 2, :], P2[:])
        y = y0 + 2 * BLK * k
        # 4 chains so 4 DMA engines carry the output in parallel
        for h0, h1 in ((0, 32), (32, 64), (64, 96), (96, 128)):
            nc.scalar.dma_start(out[n, h0:h1, y:y + 2 * BLK, :], OS[h0:h1])

    # prologue: im2col for strips 0-3, conv1 for strips 0 and 1, so the
    # steady state (conv2(s) + conv1(s+2) + im2col(s+4)) has a full
    # phase of slack on every cross-strip dependency.
    for s in (0, 1, 2, 3):
        emit_b1_alloc(s)
    emit_im2col(0)
    emit_im2col(1)
    for g in range(NG):
        emit_conv1_pair(0, g)
    emit_im2col(2)
    for g in range(NG):
        emit_conv1_pair(1, g)
    emit_im2col(3)

    # taps of im2col(s+4) issued after each conv2 block of phase s
    TAP_SCHED = [(0, 1), (2, 3), (4,), (5, 6), (7, 8), ()]

    for s in range(NSTRIPS):
        if s + 4 < NSTRIPS:
            emit_b1_alloc(s + 4)
        for k in range(NB):
            emit_conv2_block(s, k)
            if k < 5 and s + 2 < NSTRIPS:
                emit_conv1_pair(s + 2, 2 * k)
                emit_conv1_pair(s + 2, 2 * k + 1)
            if s + 4 < NSTRIPS:
                emit_im2col(s + 4, TAP_SCHED[k])


def build(mm_dt=None):
    if mm_dt is None:
        mm_dt = _mm_dt()
    nc = bacc.Bacc("TRN2", target_bir_lowering=False, debug=False,
                   num_devices=N_CORES)
    x = nc.dram_tensor("x", [NPC, C0, H0, W0], mm_dt,
                       kind="ExternalInput").ap()
    w1q = nc.dram_tensor("w1q", [27, 128], mm_dt, kind="ExternalInput").ap()
    wp = nc.dram_tensor("wp", [3, 128, C2], mm_dt, kind="ExternalInput").ap()
    ws = nc.dram_tensor("ws", [3, 128, C2], mm_dt, kind="ExternalInput").ap()
    out = nc.dram_tensor("out", [NPC, C2, H2, W2], mm_dt,
                         kind="ExternalOutput").ap()
    with tile.TileContext(nc) as tc:
        with ExitStack() as ctx:
            _emit(ctx, tc, out, x, w1q, wp, ws, mm_dt)
    nc.compile()
    return nc


def host_round(a: np.ndarray) -> np.ndarray:
    """Cast fp32 to the matmul storage dtype (bf16 cast, or tf32 rounding)."""
    a = np.ascontiguousarray(a, dtype=np.float32)
    if MODE == "bf16":
        return a.astype(ml_dtypes.bfloat16)
    b = a.view(np.uint32).copy()
    b += 0xFFF + ((b >> 13) & 1)
    b &= np.uint32(0xFFFFE000)
    return b.view(np.float32)


def pack_weights(w1: np.ndarray, w2: np.ndarray):
    """Host-side repack so every device DMA is contiguous.

    w1q[p, o] = w1[o%64, c, di, dj], p = (di*3+dj)*3 + c  (cols duplicated)
    wp[dj, k, o]: k<64 -> w2[o, k, 0, dj]; k>=64 -> w2[o, k-64, 1, dj]
    ws[dj, k, o]: k<64 -> 0;              k>=64 -> w2[o, k-64, 2, dj]
    """
    w1 = np.ascontiguousarray(np.asarray(w1), dtype=np.float32)
    w2 = np.ascontiguousarray(np.asarray(w2), dtype=np.float32)
    w1t = w1.transpose(2, 3, 1, 0).reshape(27, C1)
    w1q = np.concatenate([w1t, w1t], axis=1)
    wp = np.empty((3, 128, C2), np.float32)
    wp[:, :C1] = w2[:, :, 0, :].transpose(2, 1, 0)
    wp[:, C1:] = w2[:, :, 1, :].transpose(2, 1, 0)
    ws = np.zeros((3, 128, C2), np.float32)
    ws[:, C1:] = w2[:, :, 2, :].transpose(2, 1, 0)
    return host_round(w1q), host_round(wp), host_round(ws)


_NC_CACHE: dict = {}


def _get_nc():
    key = ("main", MODE, TY)
    if key not in _NC_CACHE:
        _NC_CACHE[key] = build()
    return _NC_CACHE[key]


def run(x, w1, w2, trace: bool = False):
    """Shard, run on 8 cores, gather.  Returns (out, BassKernelResults)."""
    x = np.ascontiguousarray(np.asarray(x), dtype=np.float32)
    assert x.shape == (FULL_N, C0, H0, W0), x.shape
    w1q, wp, ws = pack_weights(w1, w2)
    xs = host_round(x).reshape(N_CORES, NPC, C0, H0, W0)
    in_maps = [
        {"x": np.ascontiguousarray(xs[c]), "w1q": w1q, "wp": wp, "ws": ws}
        for c in range(N_CORES)
    ]
    nc = _get_nc()
    res = bass_utils.run_bass_kernel_spmd(
        nc, in_maps, core_ids=list(range(N_CORES)), trace=trace)
    out = np.concatenate(
        [np.asarray(r["out"], dtype=np.float32) for r in res.results], axis=0)
    return out, res


def kernel(x, w1, w2):
    out, _ = run(x, w1, w2, trace=False)
    return out


# revision 47
# speedup vs baseline: 1.1994x; 1.1994x over previous
"""Trainium2 Bass/Tile kernel: two chained VALID 3x3 convolutions.

    x  [N,3,256,256] --conv(w1)--> h [N,64,254,254] --conv(w2)--> out [N,128,252,252]

Data-parallel over 8 NeuronCores: batch N=16 -> 2 images per core, conv
weights replicated.

Perf structure (v5):

conv2 runs as 6 K=128 matmul passes per 2-row output chunk against a
doubled SBUF buffer H (partitions 0:64 = h, 64:128 = h shifted down one
row):

  wp[dj] @ H(t,   dj) -> taps (0,dj)+(1,dj)   (3 pair passes)
  ws[dj] @ H(t+1, dj) -> tap  (2,dj)          (3 passes, top-half weights
                                               zero so K stays 128)

Everything is tile-config (128,128), so the PE never pays the 64<->128
row-config switch the v2 kernel paid per block.

conv1 runs as 2 concurrent row-tiled K=27 matmuls (PE row groups (0,0)
and (32,0)).  The im2col buffer B1 is a flat per-partition byte stream
holding full 512 B x rows; the (di,dj) tap shift is a per-partition
byte offset, so each 3-partition load is ONE contiguous ~19.5 KB DMA
packet from HBM (38 rows of x at full width).

Copies (conv1 PSUM P1 holds h twice via column-duplicated conv1
weights, keeping everything partition-aligned):
  DVE/ACT (alternating per chunk): P1[0:64] -> H[0:64]   (the h rows)
  DMA (strip pieces):  H[0:64] rows+1 -> H[64:128]       (the row shift)
  DVE/ACT (alternating per chunk): conv2 PSUM -> OS bf16

conv2 output is staged in SBUF as bf16 and DMA'd to HBM in 6-row
pieces split into 4 per-channel-group chains (parallel DMA engines);
the host upcasts to f32.  Lookahead is one strip: phase s = conv2(s) +
conv1(s+1) bursts + im2col(s+2) loads.
"""

from contextlib import ExitStack

import ml_dtypes
import numpy as np

import concourse.bass as bass
import concourse.mybir as mybir
import concourse.tile as tile
import concourse.bass_utils as bass_utils
from concourse import bacc

N_CORES = 8
FULL_N = 16
C0, C1, C2 = 3, 64, 128

H0 = W0 = 256
H1 = W1 = 254
H2 = W2 = 252
TY = 36                                  # conv2 output rows per strip
NPC = FULL_N // N_CORES                  # images per core
SPI = H2 // TY                           # strips per image (7)
NSTRIPS = NPC * SPI                      # strips per core (14)
NC1 = (TY + 2) // 2                      # conv1 2-row chunks per strip (19)
NC2 = TY // 2                            # conv2 2-row chunks per strip (18)
BLK = 3                                  # conv2 chunks per block
NB = NC2 // BLK                          # conv2 blocks per strip (6)
NG = (NC1 + 1) // 2                      # conv1 pair-groups per strip (10)

B1_ROWS = 38                             # x rows loaded per im2col partition
B1_FLAT = 39 * W0                        # flat B1 elems/partition (spill pad)

MODE = "bf16"


def _mm_dt():
    return mybir.dt.bfloat16 if MODE == "bf16" else mybir.dt.float32r


def _emit(ctx: ExitStack, tc: tile.TileContext, out, xp, w1q, wp, ws, mm_dt):
    nc = tc.nc
    f32 = mybir.dt.float32

    wpool = ctx.enter_context(tc.tile_pool(name="weights", bufs=1))
    xpool = ctx.enter_context(tc.tile_pool(name="xst", bufs=3))
    b1pool = ctx.enter_context(tc.tile_pool(name="b1", bufs=4))
    hpool = ctx.enter_context(tc.tile_pool(name="h", bufs=3))
    opool = ctx.enter_context(tc.tile_pool(name="o2", bufs=4))
    ps1 = ctx.enter_context(tc.tile_pool(name="ps1", bufs=2, space="PSUM"))
    ps2 = ctx.enter_context(tc.tile_pool(name="ps2", bufs=4, space="PSUM"))

    # conv1 weights: [27, 128] (output cols duplicated), one copy per PE
    # 32-row group so 2 chunk matmuls run concurrently via row tiling.
    w1sb = wpool.tile([64, 128], mm_dt, tag="w1")
    for q in range(2):
        nc.sync.dma_start(w1sb[32 * q:32 * q + 27, :], w1q)
    wp_sb, ws_sb = [], []
    for dj in range(3):
        wpt = wpool.tile([128, C2], mm_dt, tag=f"wp{dj}")
        nc.sync.dma_start(wpt[:], wp[dj])
        wp_sb.append(wpt)
        wst = wpool.tile([128, C2], mm_dt, tag=f"ws{dj}")
        nc.sync.dma_start(wst[:], ws[dj])
        ws_sb.append(wst)

    def strip_of(s):
        n, k = divmod(s, SPI)
        return n, k * TY

    B1_tiles, H_tiles, X_tiles = {}, {}, {}

    def emit_b1_alloc(s):
        B1_tiles[s] = b1pool.tile([64, B1_FLAT], mm_dt, tag="b1",
                                  name=f"B1_{s}")

    def emit_xload(s):
        """Load the host-packed row-shift variants: xp[n, k, di] is
        [114, 256] = x rows y0+di..y0+di+38, channel-major (38c + r).
        HBM is read 3.4x the size of x total -- cheap -- and is served
        by only ~3 DMA engines, so everything else stays SBUF-side."""
        n, k = divmod(s, SPI)
        X_tiles[s] = []
        for di in range(3):
            XD = xpool.tile([114, W0], mm_dt, tag=f"xd{di}",
                            name=f"XD{di}_{s}")
            X_tiles[s].append(XD)
            nc.sync.dma_start(XD[:], xp[n, k, di])

    def emit_im2col(s, taps=range(9)):
        """Flat im2col: partition 32q + 3*(3di+dj) + c holds x rows
        y0+di..y0+di+38 as full 512B rows at element offset (2-dj); the
        conv1 moving AP then reads tap (di,dj) at uniform offset col 2.
        Each tap expand is ONE SBUF->SBUF dma_start whose linear
        chunk mapping sends 38 consecutive source partitions to each
        destination partition's flat stream.  Taps are spread across
        the phase's conv2 blocks (dma_start issue costs ~650ns)."""
        B1 = B1_tiles[s]
        for t9 in taps:
            di, dj = divmod(t9, 3)
            off = 2 - dj
            src = X_tiles[s][di][:]
            for q in range(2):
                p = 32 * q + 3 * t9
                nc.sync.dma_start(B1[p:p + 3, off:off + B1_ROWS * W0], src)

    def emit_conv1_pair(s, g):
        """conv1 chunks 2g..2g+1 as concurrent row-tiled K=27 matmuls
        into ONE 2-bank PSUM tile (chunk i at rows 2i, each matmul's
        512-element output staying inside one bank).  The moving AP
        takes the full 256-col window; PSUM cols 0:2 catch garbage from
        the flat layout's leading bytes and are never evicted.

        Both H halves evict from this tile at 4-row granularity:
        DVE does H[0:64] (h rows 4g..4g+4), ACT does H[64:128] at a
        -1-row offset (the row shift) -- no DMA on the critical path."""
        if g == 0:
            H_tiles[s] = hpool.tile([128, B1_ROWS, W1], mm_dt, tag="h",
                                    name=f"H{s}")
        H = H_tiles[s]
        B1v = B1_tiles[s].rearrange("p (r c) -> p r c", c=W0)
        nch = min(2, NC1 - 2 * g)
        P1 = ps1.tile([128, 4, W0], f32, tag="p1")
        for i in range(nch):
            j = 2 * g + i
            nc.tensor.matmul(
                P1[:, 2 * i:2 * i + 2, :], w1sb[32 * i:32 * i + 27, :],
                B1v[32 * i:32 * i + 27, 2 * j:2 * j + 2, :],
                start=True, stop=True, tile_position=(32 * i, 0))
        r = 4 * g
        nr = 2 * nch
        nc.vector.tensor_copy(H[0:C1, r:r + nr, :], P1[0:C1, 0:nr, 2:W0])
        if g == 0:
            nc.scalar.copy(H[C1:128, 0:nr - 1, :], P1[C1:128, 1:nr, 2:W0])
        else:
            nc.scalar.copy(H[C1:128, r - 1:r + nr - 1, :],
                           P1[C1:128, 0:nr, 2:W0])

    def emit_conv2_block(s, k):
        n, y0 = strip_of(s)
        H = H_tiles[s]
        OS = opool.tile([C2, 2 * BLK, W2], mm_dt, tag="os")
        for c in range(BLK):
            cc = BLK * k + c
            t = 2 * cc
            P2 = ps2.tile([C2, 2, W2], f32, tag="p2", name=f"P2_{c}")
            for dj in range(3):
                nc.tensor.matmul(P2[:], wp_sb[dj][:],
                                 H[:, t:t + 2, dj:dj + W2],
                                 start=(dj == 0), stop=False,
                                 skip_group_check=True)
            for dj in range(3):
                nc.tensor.matmul(P2[:], ws_sb[dj][:],
                                 H[:, t + 1:t + 3, dj:dj + W2],
                                 start=False, stop=(dj == 2),
                                 skip_group_check=True)
            if cc % 2 == 0:
                nc.vector.tensor_copy(OS[:, 2 * c:2 * c + 2, :], P2[:])
            else:
                nc.scalar.copy(OS[:, 2 * c:2 * c + 2, :], P2[:])
        y = y0 + 2 * BLK * k
        nc.sync.dma_start(out[n, :, y:y + 2 * BLK, :], OS[:])

    # prologue: im2col for strips 0-3, conv1 for strips 0 and 1, so the
    # steady state (conv2(s) + conv1(s+2) + im2col(s+4)) has a full
    # phase of slack on every cross-strip dependency.
    for s in (0, 1, 2, 3):
        emit_b1_alloc(s)
    emit_xload(0)
    emit_im2col(0)
    emit_xload(1)
    emit_im2col(1)
    for g in range(NG):
        emit_conv1_pair(0, g)
    emit_xload(2)
    emit_im2col(2)
    for g in range(NG):
        emit_conv1_pair(1, g)
    emit_xload(3)
    emit_im2col(3)
    emit_b1_alloc(4)
    emit_xload(4)

    # taps of im2col(s+4) issued after each conv2 block of phase s
    # (the XD variants for s+4 were loaded during phase s-1)
    TAP_SCHED = [(0, 1), (2, 3), (4,), (5, 6), (7, 8), ()]

    for s in range(NSTRIPS):
        if s + 5 < NSTRIPS:
            emit_b1_alloc(s + 5)
            emit_xload(s + 5)
        for k in range(NB):
            emit_conv2_block(s, k)
            if k < 5 and s + 2 < NSTRIPS:
                emit_conv1_pair(s + 2, 2 * k)
                emit_conv1_pair(s + 2, 2 * k + 1)
            if s + 4 < NSTRIPS:
                emit_im2col(s + 4, TAP_SCHED[k])


def build(mm_dt=None):
    if mm_dt is None:
        mm_dt = _mm_dt()
    nc = bacc.Bacc("TRN2", target_bir_lowering=False, debug=False,
                   num_devices=N_CORES)
    xp = nc.dram_tensor("xp", [NPC, SPI, 3, 114, W0], mm_dt,
                        kind="ExternalInput").ap()
    w1q = nc.dram_tensor("w1q", [27, 128], mm_dt, kind="ExternalInput").ap()
    wp = nc.dram_tensor("wp", [3, 128, C2], mm_dt, kind="ExternalInput").ap()
    ws = nc.dram_tensor("ws", [3, 128, C2], mm_dt, kind="ExternalInput").ap()
    out = nc.dram_tensor("out", [NPC, C2, H2, W2], mm_dt,
                         kind="ExternalOutput").ap()
    with tile.TileContext(nc) as tc:
        with ExitStack() as ctx:
            _emit(ctx, tc, out, xp, w1q, wp, ws, mm_dt)
    nc.compile()
    return nc


def host_round(a: np.ndarray) -> np.ndarray:
    """Cast fp32 to the matmul storage dtype (bf16 cast, or tf32 rounding)."""
    a = np.ascontiguousarray(a, dtype=np.float32)
    if MODE == "bf16":
        return a.astype(ml_dtypes.bfloat16)
    b = a.view(np.uint32).copy()
    b += 0xFFF + ((b >> 13) & 1)
    b &= np.uint32(0xFFFFE000)
    return b.view(np.float32)


def pack_weights(w1: np.ndarray, w2: np.ndarray):
    """Host-side repack so every device DMA is contiguous.

    w1q[p, o] = w1[o%64, c, di, dj], p = (di*3+dj)*3 + c  (cols duplicated)
    wp[dj, k, o]: k<64 -> w2[o, k, 0, dj]; k>=64 -> w2[o, k-64, 1, dj]
    ws[dj, k, o]: k<64 -> 0;              k>=64 -> w2[o, k-64, 2, dj]
    """
    w1 = np.ascontiguousarray(np.asarray(w1), dtype=np.float32)
    w2 = np.ascontiguousarray(np.asarray(w2), dtype=np.float32)
    w1t = w1.transpose(2, 3, 1, 0).reshape(27, C1)
    w1q = np.concatenate([w1t, w1t], axis=1)
    wp = np.empty((3, 128, C2), np.float32)
    wp[:, :C1] = w2[:, :, 0, :].transpose(2, 1, 0)
    wp[:, C1:] = w2[:, :, 1, :].transpose(2, 1, 0)
    ws = np.zeros((3, 128, C2), np.float32)
    ws[:, C1:] = w2[:, :, 2, :].transpose(2, 1, 0)
    return host_round(w1q), host_round(wp), host_round(ws)


_NC_CACHE: dict = {}


def _get_nc():
    key = ("main", MODE, TY)
    if key not in _NC_CACHE:
        _NC_CACHE[key] = build()
    return _NC_CACHE[key]


def pack_x(xs: np.ndarray) -> np.ndarray:
    """xp[n, k, di, 38c+r, :] = x[n, c, 36k+di+r, :] -- the three
    row-shift variants of each strip's x rows, channel-major, so every
    device-side im2col tap expand is one linear-mapped DMA."""
    npc = xs.shape[0]
    xprep = np.empty((npc, SPI, 3, 3 * B1_ROWS, W0), xs.dtype)
    for k in range(SPI):
        for di in range(3):
            y = TY * k + di
            xprep[:, k, di] = xs[:, :, y:y + B1_ROWS, :].reshape(
                npc, 3 * B1_ROWS, W0)
    return np.ascontiguousarray(xprep)


def run(x, w1, w2, trace: bool = False):
    """Shard, run on 8 cores, gather.  Returns (out, BassKernelResults)."""
    x = np.ascontiguousarray(np.asarray(x), dtype=np.float32)
    assert x.shape == (FULL_N, C0, H0, W0), x.shape
    w1q, wp, ws = pack_weights(w1, w2)
    xs = host_round(x).reshape(N_CORES, NPC, C0, H0, W0)
    in_maps = [
        {"xp": pack_x(xs[c]), "w1q": w1q, "wp": wp, "ws": ws}
        for c in range(N_CORES)
    ]
    nc = _get_nc()
    res = bass_utils.run_bass_kernel_spmd(
        nc, in_maps, core_ids=list(range(N_CORES)), trace=trace)
    out = np.concatenate(
        [np.asarray(r["out"], dtype=np.float32) for r in res.results], axis=0)
    return out, res


def kernel(x, w1, w2):
    out, _ = run(x, w1, w2, trace=False)
    return out


# revision 54
# speedup vs baseline: 1.7559x; 1.4641x over previous
"""Trainium2 Bass/Tile kernel: two chained VALID 3x3 convolutions.

    x  [N,3,256,256] --conv(w1)--> h [N,64,254,254] --conv(w2)--> out [N,128,252,252]

Data-parallel over 8 NeuronCores: batch N=16 -> 2 images per core, conv
weights replicated.  Per core the convs are computed as implicit GEMMs on the
tensor engine:

  conv1: contraction over C0*3*3=27 on SBUF partitions (im2col buffer built
         with 9 strided DMAs), one matmul per 2-row output chunk.
  conv2: contraction over C1*9=576.  h is stored doubled in SBUF: partitions
         0:64 hold h rows, partitions 64:128 the same rows shifted down by
         one.  A K=128 matmul computes a pair of row-taps (di,dj)+(di+1,dj)
         at once: 3 pairs + 3 K=64 singles = 6 matmuls per 2-row chunk.

Perf structure (v2): the PE HAM throttle holds the array at 1.2 GHz unless the
matmul stream is gap-free, so the whole kernel is a single dense PE stream:
conv1 matmuls for strip s+2 are interleaved in small bursts between conv2
chunk matmuls of strip s.  conv1 PSUM eviction runs on ScalarE, conv2
eviction on VectorE.  The h row-shift copy is done strip-at-a-time with 4
coarse SBUF->SBUF DMAs; conv2 output is staged in SBUF and written to HBM in
6-row (6 KB/partition) pieces to cut DMA packet overhead.
"""

from contextlib import ExitStack

import ml_dtypes
import numpy as np

import concourse.bass as bass
import concourse.mybir as mybir
import concourse.tile as tile
import concourse.bass_utils as bass_utils
from concourse import bacc

N_CORES = 8
FULL_N = 16
C0, C1, C2 = 3, 64, 128

MODE = "bf16"


def _mm_dt():
    return mybir.dt.bfloat16 if MODE == "bf16" else mybir.dt.float32r


def _np_dt():
    return ml_dtypes.bfloat16 if MODE == "bf16" else np.float32


class Geom:
    def __init__(self, npc, h0, w0, ty):
        self.npc = npc          # images per core
        self.h0, self.w0 = h0, w0
        self.h1, self.w1 = h0 - 2, w0 - 2
        self.h2, self.w2 = h0 - 4, w0 - 4
        self.ty = ty            # conv2 output rows per strip
        assert ty % 2 == 0 and self.h2 % ty == 0
        self.strips_per_img = self.h2 // ty
        self.nstrips = npc * self.strips_per_img
        self.nc1 = (ty + 2) // 2            # conv1 2-row chunks per strip
        self.nc2 = ty // 2                  # conv2 2-row chunks per strip


GEOM = Geom(npc=FULL_N // N_CORES, h0=256, w0=256, ty=36)

# conv2 chunks are emitted in blocks of BLK chunks; after blocks 0..len-1 a
# burst of conv1 chunks (for strip s+2) of the given size is emitted.
BLK = 3
CONV1_BURSTS = [4, 4, 4, 4, 3, 0]           # sums to nc1=19, len = nc2//BLK


def _emit(ctx: ExitStack, tc: tile.TileContext, g: Geom, out, x, w1t, w2p, w2s,
          mm_dt):
    nc = tc.nc
    f32 = mybir.dt.float32
    TY, W1, W2 = g.ty, g.w1, g.w2
    NB = g.nc2 // BLK                       # conv2 blocks per strip
    assert len(CONV1_BURSTS) == NB and sum(CONV1_BURSTS) == g.nc1

    wpool = ctx.enter_context(tc.tile_pool(name="weights", bufs=1))
    b1pool = ctx.enter_context(tc.tile_pool(name="b1", bufs=4))
    hpool = ctx.enter_context(tc.tile_pool(name="h", bufs=3))
    opool = ctx.enter_context(tc.tile_pool(name="o2", bufs=8))
    ps1 = ctx.enter_context(tc.tile_pool(name="ps1", bufs=4, space="PSUM"))
    ps2 = ctx.enter_context(tc.tile_pool(name="ps2", bufs=4, space="PSUM"))

    w1t_sb = wpool.tile([27, C1], mm_dt)
    nc.sync.dma_start(w1t_sb[:], w1t)
    # one fully-contiguous weight tile per conv2 tap column (FWL-friendly)
    w2p_sb = []
    w2s_sb = []
    for dj in range(3):
        wp = wpool.tile([128, C2], mm_dt, tag=f"w2p{dj}")
        nc.sync.dma_start(wp[:], w2p[dj])
        w2p_sb.append(wp)
        ws = wpool.tile([128, C2], mm_dt, tag=f"w2s{dj}")
        nc.sync.dma_start(ws[:], w2s[dj])
        w2s_sb.append(ws)

    def strip_of(s):
        n, k = divmod(s, g.strips_per_img)
        return n, k * TY

    B1_tiles = {}
    H_tiles = {}

    def emit_x_load(s):
        n, y0 = strip_of(s)
        B1 = b1pool.tile([27, TY + 2, W1], mm_dt, tag="b1", name=f"B1_{s}")
        B1_tiles[s] = B1
        for t9 in range(9):
            di, dj = divmod(t9, 3)
            nc.sync.dma_start(
                B1[3 * t9:3 * t9 + 3],
                x[n, :, y0 + di:y0 + di + TY + 2, dj:dj + W1])

    def emit_conv1_chunk(s, j, eng):
        """conv1 chunk j (h rows 2j:2j+2 of the strip) -> H[0:64]."""
        if j == 0:
            H_tiles[s] = hpool.tile([128, TY + 2, W1], mm_dt, tag="h",
                                    name=f"H{s}")
        H = H_tiles[s]
        B1 = B1_tiles[s]
        r = 2 * j
        P1 = ps1.tile([C1, 2, W1], f32, tag="p1")
        nc.tensor.matmul(P1[:], w1t_sb[:], B1[:, r:r + 2, :],
                         start=True, stop=True)
        if eng == "act":
            nc.scalar.copy(H[0:C1, r:r + 2, :], P1[:])
        else:
            nc.vector.tensor_copy(H[0:C1, r:r + 2, :], P1[:])

    def emit_shift(s):
        """Row-shifted copy H[64:128, r] = h row r+1, strip at a time."""
        H = H_tiles[s]
        R = TY + 1                           # 37 rows to fill
        bounds = [0, 10, 20, 29, R]
        for a, b in zip(bounds, bounds[1:]):
            nc.sync.dma_start(H[C1:128, a:b, :], H[0:C1, a + 1:b + 1, :])

    def emit_conv2_block(s, k):
        """conv2 chunks 3k..3k+2 of strip s -> staged 6-row piece -> HBM."""
        n, y0 = strip_of(s)
        H = H_tiles[s]
        OS = opool.tile([C2, 2 * BLK, W2], mm_dt, tag="os")
        # all K=128 pair matmuls of the block first, then all K=64 singles:
        # the PE pays ~90ns per 128-row <-> 64-row tile-config switch, so 2
        # switches per block instead of 2 per chunk.  Accumulation groups
        # interleave across banks (hardware-legal; skip the sim group check).
        P2s = []
        for c in range(BLK):
            t = (BLK * k + c) * 2
            P2 = ps2.tile([C2, 2, W2], f32, tag="p2", name=f"P2_{c}")
            P2s.append((t, P2))
            for dj in range(3):              # pairs: taps (0,dj)+(1,dj)
                nc.tensor.matmul(P2[:], w2p_sb[dj][:],
                                 H[:, t:t + 2, dj:dj + W2],
                                 start=(dj == 0), stop=False,
                                 skip_group_check=True)
        for c in range(BLK):
            t, P2 = P2s[c]
            for dj in range(3):
                # singles: tap (2,dj) as zero-top-padded K=128 passes on
                # H's shifted half -- same cycles as K=64 but no 64<->128
                # PE tile-config switch (~90ns each, 2 per block).
                nc.tensor.matmul(P2[:], w2s_sb[dj][:],
                                 H[:, t + 1:t + 3, dj:dj + W2],
                                 start=False, stop=(dj == 2),
                                 skip_group_check=True)
            nc.vector.tensor_copy(OS[:, 2 * c:2 * c + 2, :], P2[:])
        y = y0 + 2 * BLK * k
        nc.sync.dma_start(out[n, :, y:y + 2 * BLK, :], OS[:])

    S = g.nstrips
    # prologue: im2col for strips 0-2, conv1 for strips 0 and 1 (evictions
    # alternate DVE/ACT so the PE stream has no eviction stalls), shift(0)
    # overlapping conv1(1).
    emit_x_load(0)
    emit_x_load(1)
    emit_x_load(2)
    for j in range(g.nc1):
        emit_conv1_chunk(0, j, "act" if j % 2 else "dve")
    emit_shift(0)
    for j in range(g.nc1):
        emit_conv1_chunk(1, j, "act" if j % 2 else "dve")

    # steady state: phase s = conv2(s) with conv1(s+2) interleaved in bursts
    for s in range(S):
        if s + 3 < S:
            emit_x_load(s + 3)
        if s + 1 < S:
            emit_shift(s + 1)
        u = s + 2                            # conv1 target strip
        j0 = 0
        for k in range(NB):
            emit_conv2_block(s, k)
            if u < S:
                for j in range(j0, j0 + CONV1_BURSTS[k]):
                    emit_conv1_chunk(u, j, "act")
                j0 += CONV1_BURSTS[k]


def build(g: Geom = GEOM, mm_dt=None):
    if mm_dt is None:
        mm_dt = _mm_dt()
    nc = bacc.Bacc("TRN2", target_bir_lowering=False, debug=False,
                   num_devices=N_CORES)
    f32 = mybir.dt.float32
    x = nc.dram_tensor("x", [g.npc, C0, g.h0, g.w0], mm_dt,
                       kind="ExternalInput").ap()
    w1t = nc.dram_tensor("w1t", [27, C1], mm_dt, kind="ExternalInput").ap()
    w2p = nc.dram_tensor("w2p", [3, 128, C2], mm_dt, kind="ExternalInput").ap()
    w2s = nc.dram_tensor("w2s", [3, 128, C2], mm_dt, kind="ExternalInput").ap()
    out = nc.dram_tensor("out", [g.npc, C2, g.h2, g.w2], mm_dt,
                         kind="ExternalOutput").ap()
    with tile.TileContext(nc) as tc:
        with ExitStack() as ctx:
            _emit(ctx, tc, g, out, x, w1t, w2p, w2s, mm_dt)
    nc.compile()
    return nc


def host_round(a: np.ndarray) -> np.ndarray:
    """Cast fp32 to the matmul storage dtype (bf16 cast, or tf32 rounding)."""
    a = np.ascontiguousarray(a, dtype=np.float32)
    if MODE == "bf16":
        return a.astype(ml_dtypes.bfloat16)
    b = a.view(np.uint32).copy()
    b += 0xFFF + ((b >> 13) & 1)
    b &= np.uint32(0xFFFFE000)
    return b.view(np.float32)


def pack_weights(w1: np.ndarray, w2: np.ndarray):
    """Host-side repack so every device DMA is contiguous.

    w1t[p, o] = w1[o, c, di, dj] with p = (di*3+dj)*3 + c  (matches im2col)
    w2p[dj, k, o]: k<64 -> w2[o, k, 0, dj]; k>=64 -> w2[o, k-64, 1, dj]
    w2s[dj, c, o] = w2[o, c, 2, dj]
    """
    w1 = np.ascontiguousarray(np.asarray(w1), dtype=np.float32)
    w2 = np.ascontiguousarray(np.asarray(w2), dtype=np.float32)
    w1t = np.ascontiguousarray(w1.transpose(2, 3, 1, 0).reshape(27, C1))
    w2p = np.empty((3, 128, C2), np.float32)
    w2p[:, :C1] = w2[:, :, 0, :].transpose(2, 1, 0)
    w2p[:, C1:] = w2[:, :, 1, :].transpose(2, 1, 0)
    w2s = np.zeros((3, 128, C2), np.float32)
    w2s[:, C1:] = w2[:, :, 2, :].transpose(2, 1, 0)
    return host_round(w1t), host_round(w2p), host_round(w2s)


_NC_CACHE: dict = {}


def _get_nc():
    key = ("main", MODE, GEOM.ty)
    if key not in _NC_CACHE:
        _NC_CACHE[key] = build()
    return _NC_CACHE[key]


def run(x, w1, w2, trace: bool = False):
    """Shard, run on 8 cores, gather.  Returns (out, BassKernelResults)."""
    x = np.ascontiguousarray(np.asarray(x), dtype=np.float32)
    assert x.shape == (FULL_N, C0, GEOM.h0, GEOM.w0), x.shape
    w1t, w2p, w2s = pack_weights(w1, w2)
    xs = host_round(x).reshape(N_CORES, GEOM.npc, C0, GEOM.h0, GEOM.w0)
    in_maps = [
        {"x": np.ascontiguousarray(xs[c]), "w1t": w1t, "w2p": w2p, "w2s": w2s}
        for c in range(N_CORES)
    ]
    nc = _get_nc()
    res = bass_utils.run_bass_kernel_spmd(
        nc, in_maps, core_ids=list(range(N_CORES)), trace=trace)
    out = np.concatenate(
        [np.asarray(r["out"], dtype=np.float32) for r in res.results], axis=0)
    return out, res


def kernel(x, w1, w2):
    out, _ = run(x, w1, w2, trace=False)
    return out

